# revision 1
# baseline (speedup 1.0000x reference)
"""BLT local encoder on 8 trn2 NeuronCores (Bass/Tile SPMD).

Sharding: 8 cores = 4 batches x 2 "parity" halves. Core (b, p) owns the
64-token chunks of batch b with chunk parity p (1024 tokens, load-balanced
causal attention) and patches [256p, 256p+256).

Per-core pipeline (fp32):
  hash-embed gather (indirect DMA + CCE add) -> rms -> QKV (+fused RoPE on
  deinterleaved heads) -> pair AllGather of K(D-major)/V' -> causal
  attention in score-transposed layout (k on partitions) with softmax
  denominators via an appended ones-column of V' (scores are O(1), so exp
  without max-subtraction) -> out-proj + residual -> rms -> SwiGLU FFN ->
  pair AllGather of h -> patch amax-pool via indirect gather -> cross
  attention (patch queries over byte keys) -> out.

Self-contained: shapes hardcoded for
B,S,P,D,NH,DH,F = 4,2048,512,512,8,64,1536, HASH_VOCAB=50002, BYTE_VOCAB=260.
"""
import math
import numpy as np
import ml_dtypes

BF = ml_dtypes.bfloat16

import concourse.bass as bass
import concourse.mybir as mybir
import concourse.tile as tile
from concourse import bacc
from concourse.bass_utils import run_bass_kernel_spmd
from concourse.masks import make_identity

# ----- problem constants (must match reference.py) -----
B, S, P = 4, 2048, 512
D, NH, DH, F = 512, 8, 64, 1536
BYTE_VOCAB = 260
HASH_VOCAB = 50002
GROUP_SIZES = (3, 4)
PRIMES = (1000000007, 5915587277)
N_TABLES = 4
TBL_ROWS = BYTE_VOCAB + N_TABLES * HASH_VOCAB  # 200268
NCORES = 8
T = S // 2            # own tokens per core
NT = T // 128         # 8 token tiles
DC = D // 128         # 4 D chunks
FC = F // 128         # 12 F chunks
PP = P // 2           # own patches per core (256)
NPC = PP // 128       # 2 patch tiles
HALF = DH // 2        # 32
VW = NH * (DH + 1)    # 520
EPS = 1e-5

fp32 = mybir.dt.float32
bf16 = mybir.dt.bfloat16
i32 = mybir.dt.int32
AF = mybir.ActivationFunctionType
OP = mybir.AluOpType


# ================= host-side preparation (numpy only) =================

def _hash_indices(tokens):
    Bt, St = tokens.shape
    out = np.zeros((N_TABLES, Bt, St), np.int64)
    idx = 0
    for prime in PRIMES:
        p = prime % HASH_VOCAB
        for g in GROUP_SIZES:
            xp = np.concatenate([np.zeros((Bt, g - 1), tokens.dtype), tokens], 1)
            hsh = np.zeros((Bt, St), np.int64)
            pw = 1
            for i in range(g):
                hsh = (hsh + xp[:, i:i + St].astype(np.int64) * pw) % HASH_VOCAB
                pw = (pw * p) % HASH_VOCAB
            out[idx] = BYTE_VOCAB + idx * HASH_VOCAB + hsh
            idx += 1
    return out


def _own_tokens(p):
    chunks = np.arange(16) * 2 + p
    return (chunks[:, None] * 64 + np.arange(64)[None, :]).reshape(-1)


def _perm_head_deint():
    perm = np.zeros(D, np.int64)
    for h in range(NH):
        base = 64 * h
        perm[base:base + HALF] = base + 2 * np.arange(HALF)
        perm[base + HALF:base + DH] = base + 2 * np.arange(HALF) + 1
    return perm


def _rope_tables(own_pos):
    freqs = 1.0 / (10000.0 ** (np.arange(HALF, dtype=np.float64) / HALF))
    cos = np.zeros((128, NT * 256), np.float32)
    sin = np.zeros((128, NT * 256), np.float32)
    for tt in range(NT):
        pos = own_pos[128 * tt:128 * tt + 128].astype(np.float64)
        ang = pos[:, None] * freqs[None, :]
        cos[:, 256 * tt:256 * (tt + 1)] = np.tile(np.cos(ang), (1, NH)).astype(np.float32)
        sin[:, 256 * tt:256 * (tt + 1)] = np.tile(np.sin(ang), (1, NH)).astype(np.float32)
    return cos, sin


def _attn_masks(p):
    # slot 0 = local block (delta 0), slot 1 = remote block (delta 64*((1-p)-p))
    rel = np.arange(128)
    rel = np.where(rel < 64, rel, rel + 64)
    out = np.zeros((128, 256), np.float32)
    for slot, delta in ((0, 0), (1, 64 * ((1 - p) - p))):
        m = (rel[:, None] + delta) <= rel[None, :]
        out[:, 128 * slot:128 * (slot + 1)] = m.astype(np.float32)
    return out


def _remote_idx(p):
    # [128, 12] int32: cols 0..3 -> remote k/ck rows per D-chunk c;
    # cols 4..11 -> remote v/cv/h3 rows per token tile j
    r = np.arange(128)
    out = np.zeros((128, 12), np.int32)
    for c in range(DC):
        out[:, c] = (1 - p) * 512 + 128 * c + r
    for j in range(NT):
        out[:, 4 + j] = (1 - p) * 1024 + 128 * j + r
    return out


def _ag_pos(g):
    ch = g // 64
    return 1024 * (ch % 2) + 64 * (ch // 2) + g % 64


def _pool_indices(pl_b, p):
    cum = np.cumsum(pl_b)
    starts = np.concatenate([[0], cum[:-1]])
    ends = cum
    pgidx = np.zeros((128, 16), np.int32)
    pmask = np.zeros((128, 2), np.float32)
    for pc in range(NPC):
        for r in range(128):
            patch = 256 * p + 128 * pc + r
            st, en = int(starts[patch]), int(min(ends[patch], S))
            if st >= S or en <= st:
                pmask[r, pc] = 0.0
            else:
                sl = np.minimum(st + np.arange(8), en - 1)
                pgidx[r, 8 * pc:8 * pc + 8] = [_ag_pos(int(x)) for x in sl]
                pmask[r, pc] = 1.0
    return pgidx, pmask


def prepare_inputs(inputs):
    tokens = np.asarray(inputs["tokens"])
    pl = np.asarray(inputs["patch_lengths"])
    tok_emb = np.asarray(inputs["tok_emb"], np.float32)
    hash_emb = np.asarray(inputs["hash_emb"], np.float32)

    tables = np.ascontiguousarray(
        np.concatenate([tok_emb, hash_emb.reshape(-1, D)], 0)).astype(BF)
    assert tables.shape == (TBL_ROWS, D)
    hidx = _hash_indices(tokens)

    perm = _perm_head_deint()
    wq = np.ascontiguousarray(
        np.asarray(inputs["wq"], np.float32)[:, perm] * (1.0 / math.sqrt(DH)))
    wk = np.ascontiguousarray(np.asarray(inputs["wk"], np.float32)[:, perm])
    cwq = np.ascontiguousarray(
        np.asarray(inputs["cwq"], np.float32) * (1.0 / math.sqrt(DH)))

    shared = {
        "tables": tables, "wq": wq.astype(BF), "wk": wk.astype(BF),
        "wv": np.asarray(inputs["wv"], np.float32).astype(BF),
        "wo": np.asarray(inputs["wo"], np.float32).astype(BF),
        "w1": np.asarray(inputs["w1"], np.float32).astype(BF),
        "w3": np.asarray(inputs["w3"], np.float32).astype(BF),
        "w2": np.asarray(inputs["w2"], np.float32).astype(BF),
        "cwq": cwq.astype(BF),
        "cwk": np.asarray(inputs["cwk"], np.float32).astype(BF),
        "cwv": np.asarray(inputs["cwv"], np.float32).astype(BF),
        "cwo": np.asarray(inputs["cwo"], np.float32).astype(BF),
    }

    in_maps = []
    for core in range(NCORES):
        b, p = core // 2, core % 2
        own = _own_tokens(p)
        gidx = np.zeros((128, 5 * NT), np.int32)
        for tt in range(NT):
            g = own[128 * tt:128 * tt + 128]
            gidx[:, 5 * tt] = tokens[b, g]
            for s in range(N_TABLES):
                gidx[:, 5 * tt + s + 1] = hidx[s, b, g]
        cos, sin = _rope_tables(own)
        pgidx, pmask = _pool_indices(pl[b], p)
        m = dict(shared)
        m.update({"gidx": gidx, "ropecos": cos, "ropesin": sin,
                  "masks": _attn_masks(p).astype(BF), "pgidx": pgidx, "pmask": pmask,
                  "ridx": _remote_idx(p)})
        in_maps.append(m)
    return in_maps


def assemble_output(results):
    out = np.zeros((B, P, D), np.float32)
    for core in range(NCORES):
        b, p = core // 2, core % 2
        out[b, 256 * p:256 * (p + 1), :] = results[core]["out"]
    return out


# ========================= kernel build =========================

def _rms_tiles(nc, small, scratch, src_tiles, dst_tiles, n, eps_ap=None):
    for t in range(n):
        ssq = small.tile([128, 1], fp32, tag="ssq", name="ssq")
        nc.vector.scalar_tensor_tensor(
            out=dst_tiles[t][:], in0=src_tiles[t][:], scalar=1.0,
            in1=src_tiles[t][:], op0=OP.mult, op1=OP.mult, accum_out=ssq[:])
        sq = small.tile([128, 1], fp32, tag="sqm", name="sqm")
        nc.scalar.activation(sq[:], ssq[:], AF.Sqrt, bias=eps_ap, scale=1.0 / D)
        rs = small.tile([128, 1], fp32, tag="rs", name="rs")
        nc.vector.reciprocal_approx_fast(rs[:], sq[:])
        nc.vector.tensor_scalar_mul(dst_tiles[t][:], src_tiles[t][:], rs[:, 0:1])


def _transpose_to_dm(nc, tc, tm_tiles, dm_tiles, ident, ntiles, width=D):
    """tm_tiles: ntiles x [128, width]; dm_tiles: width/128 x [128, 128*ntiles]."""
    trp = tc.alloc_tile_pool(name="trp", bufs=2, space="PSUM")
    for c in range(width // 128):
        for t in range(ntiles):
            tp = trp.tile([128, 128], bf16, tag="trp", name="trp")
            nc.tensor.transpose(out=tp[:], in_=tm_tiles[t][:, 128 * c:128 * (c + 1)],
                                identity=ident[:])
            nc.vector.tensor_copy(dm_tiles[c][:, 128 * t:128 * (t + 1)], tp[:])
    trp.release()


def _head_view(ap2d, h=NH):
    return ap2d.rearrange("p (h f) -> p h f", h=h)


def build_nc():
    nc = bacc.Bacc("TRN2", target_bir_lowering=False, debug=False,
                   num_devices=NCORES)

    # ---- I/O ----
    tables = nc.dram_tensor("tables", [TBL_ROWS, D], bf16, kind="ExternalInput").ap()
    gidx_d = nc.dram_tensor("gidx", [128, 5 * NT], i32, kind="ExternalInput").ap()
    w_d = {}
    for nm in ("wq", "wk", "wv", "wo", "cwq", "cwk", "cwv", "cwo"):
        w_d[nm] = nc.dram_tensor(nm, [D, D], bf16, kind="ExternalInput").ap()
    w1_d = nc.dram_tensor("w1", [D, F], bf16, kind="ExternalInput").ap()
    w3_d = nc.dram_tensor("w3", [D, F], bf16, kind="ExternalInput").ap()
    w2_d = nc.dram_tensor("w2", [F, D], bf16, kind="ExternalInput").ap()
    cos_d = nc.dram_tensor("ropecos", [128, NT * 256], fp32, kind="ExternalInput").ap()
    sin_d = nc.dram_tensor("ropesin", [128, NT * 256], fp32, kind="ExternalInput").ap()
    masks_d = nc.dram_tensor("masks", [128, 256], bf16, kind="ExternalInput").ap()
    pgidx_d = nc.dram_tensor("pgidx", [128, 16], i32, kind="ExternalInput").ap()
    pmask_d = nc.dram_tensor("pmask", [128, 2], fp32, kind="ExternalInput").ap()
    ridx_d = nc.dram_tensor("ridx", [128, 12], i32, kind="ExternalInput").ap()
    out_d = nc.dram_tensor("out", [PP, D], fp32, kind="ExternalOutput").ap()

    # ---- internal DRAM (collectives / gather source) ----
    kag_in = nc.dram_tensor("kag_in", [D, T], bf16).ap()
    kag_out = nc.dram_tensor("kag_out", [2 * D, T], bf16).ap()
    vag_in = nc.dram_tensor("vag_in", [T, VW], bf16).ap()
    vag_out = nc.dram_tensor("vag_out", [S, VW], bf16).ap()
    h3_in = nc.dram_tensor("h3_in", [T, D], bf16).ap()
    h3_full = nc.dram_tensor("h3_full", [S, D], bf16).ap()
    ckag_in = nc.dram_tensor("ckag_in", [D, T], bf16).ap()
    ckag_out = nc.dram_tensor("ckag_out", [2 * D, T], bf16).ap()
    cvag_in = nc.dram_tensor("cvag_in", [T, VW], bf16).ap()
    cvag_out = nc.dram_tensor("cvag_out", [S, VW], bf16).ap()

    groups = [[2 * i, 2 * i + 1] for i in range(NCORES // 2)]

    with tile.TileContext(nc) as tc:
        const = tc.alloc_tile_pool(name="const", bufs=1, side="left")
        persist = tc.alloc_tile_pool(name="persist", bufs=1, side="left")
        small = tc.alloc_tile_pool(name="small", bufs=2, side="left")
        scratch = tc.alloc_tile_pool(name="scratch", bufs=2, side="left")
        epool = tc.alloc_tile_pool(name="epool", bufs=4, side="left")

        qdm_pool = tc.alloc_tile_pool(name="qdm_pool", bufs=1, side="right")
        kdm_pool = tc.alloc_tile_pool(name="kdm_pool", bufs=1, side="right")
        qktm_pool = tc.alloc_tile_pool(name="qktm", bufs=1, side="right")
        hnT_pool = tc.alloc_tile_pool(name="hnT_pool", bufs=1, side="right")
        ropes = tc.alloc_tile_pool(name="ropes", bufs=2, side="right")
        wqkv = tc.alloc_tile_pool(name="wqkv", bufs=2, side="right")
        rtmp = tc.alloc_tile_pool(name="rtmp", bufs=2, side="right")

        # constants
        ident = const.tile([128, 128], bf16)
        make_identity(nc, ident[:])
        gidx = const.tile([128, 5 * NT], i32)
        nc.sync.dma_start(gidx[:], gidx_d[:])
        masks_sb = const.tile([128, 256], bf16)
        nc.sync.dma_start(masks_sb[:], masks_d[:])
        pgidx_sb = const.tile([128, 16], i32)
        nc.sync.dma_start(pgidx_sb[:], pgidx_d[:])
        pmask_sb = const.tile([128, 2], fp32)
        nc.sync.dma_start(pmask_sb[:], pmask_d[:])
        eps_sb = const.tile([128, 1], fp32)
        nc.gpsimd.memset(eps_sb[:], EPS)
        ridx_sb = const.tile([128, 12], i32)
        nc.sync.dma_start(ridx_sb[:], ridx_d[:])

        # ---------- A: embedding gather-sum (5 parallel gathers + add tree) ----------
        h = [persist.tile([128, D], fp32, tag=f"h{t}", name=f"h{t}") for t in range(NT)]
        gpool = tc.alloc_tile_pool(name="gpool", bufs=10, side="right")
        for t in range(NT):
            gts = []
            for s in range(5):
                gt = gpool.tile([128, D], bf16, tag="emb", name="emb")
                nc.gpsimd.indirect_dma_start(
                    out=gt[:], out_offset=None, in_=tables[:],
                    in_offset=bass.IndirectOffsetOnAxis(
                        ap=gidx[:, 5 * t + s:5 * t + s + 1], axis=0))
                gts.append(gt)
            a = gpool.tile([128, D], fp32, tag="emba", name="emba")
            b = gpool.tile([128, D], fp32, tag="embb", name="embb")
            nc.vector.tensor_tensor(out=a[:], in0=gts[0][:], in1=gts[1][:], op=OP.add)
            nc.vector.tensor_tensor(out=b[:], in0=gts[2][:], in1=gts[3][:], op=OP.add)
            nc.vector.tensor_tensor(out=a[:], in0=a[:], in1=b[:], op=OP.add)
            nc.vector.tensor_tensor(out=h[t][:], in0=a[:], in1=gts[4][:], op=OP.add)
        gpool.release()

        # ---------- B: rms1 + hnT ----------
        hn = [persist.tile([128, D], bf16, tag=f"hn{t}", name=f"hn{t}") for t in range(NT)]
        _rms_tiles(nc, small, scratch, h, hn, NT, eps_sb[:, 0:1])
        hnT = [hnT_pool.tile([128, T], bf16, tag=f"hnT{c}", name=f"hnT{c}") for c in range(DC)]
        _transpose_to_dm(nc, tc, hn, hnT, ident, NT)

        # ---------- C: q, k (rope), v' ----------
        cps = tc.alloc_tile_pool(name="cps", bufs=2, space="PSUM", side="right")

        q_tm = [qktm_pool.tile([128, D], bf16, tag=f"qtm{t}", name=f"qtm{t}") for t in range(NT)]
        k_tm = [qktm_pool.tile([128, D], bf16, tag=f"ktm{t}", name=f"ktm{t}") for t in range(NT)]
        v_sb = [persist.tile([128, VW], bf16, tag=f"v{t}", name=f"v{t}") for t in range(NT)]

        cos_all = ropes.tile([128, NT * 256], fp32, tag="cosall", name="cosall", bufs=1)
        nc.sync.dma_start(cos_all[:], cos_d[:])
        sin_all = ropes.tile([128, NT * 256], fp32, tag="sinall", name="sinall", bufs=1)
        nc.sync.dma_start(sin_all[:], sin_d[:])

        for name, dst in (("wk", k_tm),):
            w_sb = [wqkv.tile([128, D], bf16, tag=f"w{c}", name=f"{name}{c}") for c in range(DC)]
            for c in range(DC):
                nc.sync.dma_start(w_sb[c][:], w_d[name][128 * c:128 * (c + 1), :])
            for t in range(NT):
                ps = cps.tile([128, D], fp32, tag="qkvp", name="qkvp")
                for c in range(DC):
                    nc.tensor.matmul(ps[:], hnT[c][:, 128 * t:128 * (t + 1)],
                                     w_sb[c][:], start=(c == 0), stop=(c == DC - 1))
                cosv = _head_view(cos_all[:, 256 * t:256 * (t + 1)])
                sinv = _head_view(sin_all[:, 256 * t:256 * (t + 1)])
                psv = ps[:].rearrange("p (h y) -> p h y", h=NH)
                x1, x2 = psv[:, :, 0:HALF], psv[:, :, HALF:DH]
                dv = dst[t][:].rearrange("p (h y) -> p h y", h=NH)
                o1, o2 = dv[:, :, 0:HALF], dv[:, :, HALF:DH]
                t1 = _head_view(rtmp.tile([128, 256], fp32, tag="t1", name="t1")[:])
                t2 = _head_view(rtmp.tile([128, 256], fp32, tag="t2", name="t2")[:])
                t3 = _head_view(rtmp.tile([128, 256], fp32, tag="t3", name="t3")[:])
                t4 = _head_view(rtmp.tile([128, 256], fp32, tag="t4", name="t4")[:])
                nc.vector.tensor_tensor(out=t1, in0=x1, in1=cosv, op=OP.mult)
                nc.vector.tensor_tensor(out=t2, in0=x2, in1=sinv, op=OP.mult)
                nc.vector.tensor_tensor(out=o1, in0=t1, in1=t2, op=OP.subtract)
                nc.vector.tensor_tensor(out=t3, in0=x1, in1=sinv, op=OP.mult)
                nc.vector.tensor_tensor(out=t4, in0=x2, in1=cosv, op=OP.mult)
                nc.vector.tensor_tensor(out=o2, in0=t3, in1=t4, op=OP.add)

        k_dm = [kdm_pool.tile([128, T], bf16, tag=f"kdm{c}", name=f"kdm{c}") for c in range(DC)]
        _transpose_to_dm(nc, tc, k_tm, k_dm, ident, NT)
        for c in range(DC):
            nc.sync.dma_start(kag_in[128 * c:128 * (c + 1), :], k_dm[c][:])
        nc.gpsimd.collective_compute("AllGather", OP.bypass, replica_groups=groups,
                                     ins=[kag_in[:]], outs=[kag_out[:]])

        wv_sb = [wqkv.tile([128, D], bf16, tag=f"w{c}", name=f"wv{c}") for c in range(DC)]
        for c in range(DC):
            nc.sync.dma_start(wv_sb[c][:], w_d["wv"][128 * c:128 * (c + 1), :])
        for t in range(NT):
            ps = cps.tile([128, D], fp32, tag="qkvp", name="qkvp")
            for c in range(DC):
                nc.tensor.matmul(ps[:], hnT[c][:, 128 * t:128 * (t + 1)],
                                 wv_sb[c][:], start=(c == 0), stop=(c == DC - 1))
            vv = v_sb[t][:].rearrange("p (h e) -> p h e", h=NH)
            nc.vector.tensor_copy(out=vv[:, :, 0:DH], in_=_head_view(ps[:], h=NH))
            nc.gpsimd.memset(vv[:, :, DH:DH + 1], 1.0)
        for t in range(NT):
            nc.sync.dma_start(vag_in[128 * t:128 * (t + 1), :], v_sb[t][:])
        nc.gpsimd.collective_compute("AllGather", OP.bypass, replica_groups=groups,
                                     ins=[vag_in[:]], outs=[vag_out[:]])

        for name, dst in (("wq", q_tm),):
            w_sb = [wqkv.tile([128, D], bf16, tag=f"w{c}", name=f"{name}{c}") for c in range(DC)]
            for c in range(DC):
                nc.sync.dma_start(w_sb[c][:], w_d[name][128 * c:128 * (c + 1), :])
            for t in range(NT):
                ps = cps.tile([128, D], fp32, tag="qkvp", name="qkvp")
                for c in range(DC):
                    nc.tensor.matmul(ps[:], hnT[c][:, 128 * t:128 * (t + 1)],
                                     w_sb[c][:], start=(c == 0), stop=(c == DC - 1))
                cosv = _head_view(cos_all[:, 256 * t:256 * (t + 1)])
                sinv = _head_view(sin_all[:, 256 * t:256 * (t + 1)])
                psv = ps[:].rearrange("p (h y) -> p h y", h=NH)
                x1, x2 = psv[:, :, 0:HALF], psv[:, :, HALF:DH]
                dv = dst[t][:].rearrange("p (h y) -> p h y", h=NH)
                o1, o2 = dv[:, :, 0:HALF], dv[:, :, HALF:DH]
                t1 = _head_view(rtmp.tile([128, 256], fp32, tag="t1", name="t1")[:])
                t2 = _head_view(rtmp.tile([128, 256], fp32, tag="t2", name="t2")[:])
                t3 = _head_view(rtmp.tile([128, 256], fp32, tag="t3", name="t3")[:])
                t4 = _head_view(rtmp.tile([128, 256], fp32, tag="t4", name="t4")[:])
                nc.vector.tensor_tensor(out=t1, in0=x1, in1=cosv, op=OP.mult)
                nc.vector.tensor_tensor(out=t2, in0=x2, in1=sinv, op=OP.mult)
                nc.vector.tensor_tensor(out=o1, in0=t1, in1=t2, op=OP.subtract)
                nc.vector.tensor_tensor(out=t3, in0=x1, in1=sinv, op=OP.mult)
                nc.vector.tensor_tensor(out=t4, in0=x2, in1=cosv, op=OP.mult)
                nc.vector.tensor_tensor(out=o2, in0=t3, in1=t4, op=OP.add)

        # ---------- D: q -> D-major ----------
        q_dm = [qdm_pool.tile([128, T], bf16, tag=f"qdm{c}", name=f"qdm{c}") for c in range(DC)]
        _transpose_to_dm(nc, tc, q_tm, q_dm, ident, NT)
        cps.release()
        rtmp.release()
        wqkv.release()
        ropes.release()
        hnT_pool.release()
        qktm_pool.release()

        # ---------- E: self attention (local half overlaps the k/v AllGather) ----------
        vbr_pool = tc.alloc_tile_pool(name="vbr_pool", bufs=1, side="right")
        kbr_pool = tc.alloc_tile_pool(name="kbr_pool", bufs=2, side="right")
        vbr = [vbr_pool.tile([128, VW], bf16, tag=f"vbr{j}", name=f"vbr{j}")
               for j in range(NT)]
        for j in range(NT):
            nc.gpsimd.indirect_dma_start(
                out=vbr[j][:], out_offset=None, in_=vag_out[:],
                in_offset=bass.IndirectOffsetOnAxis(
                    ap=ridx_sb[:, 4 + j:5 + j], axis=0))

        att_ps = tc.alloc_tile_pool(name="att_ps", bufs=4, space="PSUM", side="right")
        oT_ps = tc.alloc_tile_pool(name="oT_ps", bufs=4, space="PSUM", side="right")
        odm_pool = tc.alloc_tile_pool(name="odm_pool", bufs=1, side="right")
        o_dm = [odm_pool.tile([128, T], bf16, tag=f"odm{c}", name=f"odm{c}") for c in range(DC)]

        kbr = None
        for hp in range(4):
            c = hp
            kbr = kbr_pool.tile([128, T], bf16, tag="kbr", name="kbr")
            nc.gpsimd.indirect_dma_start(
                out=kbr[:], out_offset=None, in_=kag_out[:],
                in_offset=bass.IndirectOffsetOnAxis(
                    ap=ridx_sb[:, c:c + 1], axis=0))
            oT = [[oT_ps.tile([65, 512], fp32, tag="oT", name="oT")
                   for g in range(2)] for hf in range(2)]
            started = [[False, False], [False, False]]
            for src in range(2):
                for j in range(NT):
                    for hf in range(2):
                        hh = 2 * hp + hf
                        hrow = 64 * hf
                        if src == 0:
                            lhs_k = k_dm[c][hrow:hrow + 64, 128 * j:128 * (j + 1)]
                            vt = v_sb[j]
                        else:
                            lhs_k = kbr[hrow:hrow + 64, 128 * j:128 * (j + 1)]
                            vt = vbr[j]
                        lhs_v = vt[:, (DH + 1) * hh:(DH + 1) * hh + DH + 1]
                        for g in range(2):
                            q0, q1 = 512 * g, 512 * (g + 1)
                            s0 = max(128 * j, q0)
                            if s0 >= q1:
                                continue
                            n = q1 - s0
                            sc = att_ps.tile([128, 512], fp32, tag="sc", name="sc")
                            nc.tensor.matmul(sc[:, 0:n], lhs_k,
                                             q_dm[c][hrow:hrow + 64, s0:q1],
                                             start=True, stop=True)
                            e = epool.tile([128, 512], bf16, tag="expT", name="expT")
                            nc.scalar.activation(e[:, 0:n], sc[:, 0:n], AF.Exp)
                            if g == j // 4:
                                nc.vector.tensor_tensor(
                                    out=e[:, 0:128], in0=e[:, 0:128],
                                    in1=masks_sb[:, 128 * src:128 * (src + 1)],
                                    op=OP.mult)
                            last = (src == 1) and ((g == 0 and j == 3) or
                                                   (g == 1 and j == NT - 1))
                            nc.tensor.matmul(oT[hf][g][:, s0 - q0:s0 - q0 + n],
                                             lhs_v, e[:, 0:n],
                                             start=(not started[hf][g]), stop=last)
                            started[hf][g] = True
            for hf in range(2):
                hrow = 64 * hf
                for g in range(2):
                    rcp = small.tile([1, 512], fp32, tag="rcp", name="rcp")
                    nc.scalar.copy(rcp[:], oT[hf][g][64:65, :])
                    rsum = small.tile([1, 512], fp32, tag="rsum", name="rsum")
                    nc.vector.reciprocal_approx_fast(rsum[:], rcp[:])
                    rbc = scratch.tile([64, 512], fp32, tag="rbc", name="rbc")
                    nc.gpsimd.partition_broadcast(rbc[:], rsum[:], channels=64)
                    nc.vector.tensor_tensor(
                        out=o_dm[c][hrow:hrow + 64, 512 * g:512 * (g + 1)],
                        in0=oT[hf][g][0:64, :], in1=rbc[:], op=OP.mult)
        oT_ps.release()
        att_ps.release()

        # ---------- F: out-proj + residual ----------
        wos = tc.alloc_tile_pool(name="wos", bufs=1, side="right")
        wops = tc.alloc_tile_pool(name="wops", bufs=2, space="PSUM", side="right")
        wo_sb = [wos.tile([128, D], bf16, tag=f"wo{c}", name=f"wo{c}") for c in range(DC)]
        for c in range(DC):
            nc.sync.dma_start(wo_sb[c][:], w_d["wo"][128 * c:128 * (c + 1), :])
        for t in range(NT):
            ps = wops.tile([128, D], fp32, tag="wop", name="wop")
            for c in range(DC):
                nc.tensor.matmul(ps[:], o_dm[c][:, 128 * t:128 * (t + 1)],
                                 wo_sb[c][:], start=(c == 0), stop=(c == DC - 1))
            nc.vector.tensor_tensor(out=h[t][:], in0=h[t][:], in1=ps[:], op=OP.add)
        wops.release()
        wos.release()
        odm_pool.release()
        kbr_pool.release()
        vbr_pool.release()
        kdm_pool.release()
        qdm_pool.release()

        # ---------- G: rms2 + FFN ----------
        hn2 = hn
        _rms_tiles(nc, small, scratch, h, hn2, NT, eps_sb[:, 0:1])
        hn2T_pool = tc.alloc_tile_pool(name="hn2T_pool", bufs=1, side="right")
        hn2T = [hn2T_pool.tile([128, T], bf16, tag=f"hn2T{c}", name=f"hn2T{c}") for c in range(DC)]
        _transpose_to_dm(nc, tc, hn2, hn2T, ident, NT)

        wf = tc.alloc_tile_pool(name="wf", bufs=1, side="right")
        ffn_ps = tc.alloc_tile_pool(name="ffn_ps", bufs=2, space="PSUM", side="right")
        mid_pool = tc.alloc_tile_pool(name="mid_pool", bufs=1, side="right")
        mid = [mid_pool.tile([128, T], bf16, tag=f"mid{f}", name=f"mid{f}") for f in range(FC)]
        w1_sb = [wf.tile([128, F], bf16, tag=f"w1s{c}", name=f"w1s{c}", bufs=1) for c in range(DC)]
        w3_sb = [wf.tile([128, F], bf16, tag=f"w3s{c}", name=f"w3s{c}", bufs=1) for c in range(DC)]
        for c in range(DC):
            nc.sync.dma_start(w1_sb[c][:], w1_d[128 * c:128 * (c + 1), :])
            nc.sync.dma_start(w3_sb[c][:], w3_d[128 * c:128 * (c + 1), :])
        for f in range(FC):
            fs = slice(128 * f, 128 * (f + 1))
            for g in range(2):
                tr = slice(512 * g, 512 * (g + 1))
                gp = ffn_ps.tile([128, 512], fp32, tag="gp", name="gp")
                up = ffn_ps.tile([128, 512], fp32, tag="up", name="up")
                for c in range(DC):
                    nc.tensor.matmul(gp[:], w1_sb[c][:, fs], hn2T[c][:, tr],
                                     start=(c == 0), stop=(c == DC - 1))
                for c in range(DC):
                    nc.tensor.matmul(up[:], w3_sb[c][:, fs], hn2T[c][:, tr],
                                     start=(c == 0), stop=(c == DC - 1))
                gs = scratch.tile([128, 512], fp32, tag="gs", name="gs")
                nc.scalar.activation(gs[:], gp[:], AF.Silu)
                nc.vector.tensor_tensor(out=mid[f][:, tr], in0=gs[:], in1=up[:],
                                        op=OP.mult)
        w2_sb = [wf.tile([128, D], bf16, tag=f"w2s{f}", name=f"w2s{f}", bufs=1) for f in range(FC)]
        for f in range(FC):
            nc.sync.dma_start(w2_sb[f][:], w2_d[128 * f:128 * (f + 1), :])
        for t in range(NT):
            ps = ffn_ps.tile([128, D], fp32, tag="dp", name="dp")
            for f in range(FC):
                nc.tensor.matmul(ps[:], mid[f][:, 128 * t:128 * (t + 1)],
                                 w2_sb[f][:], start=(f == 0), stop=(f == FC - 1))
            nc.vector.tensor_tensor(out=h[t][:], in0=h[t][:], in1=ps[:], op=OP.add)
        mid_pool.release()
        ffn_ps.release()
        wf.release()

        # ---------- H: h3 AllGather, kvn, ck/cv + AllGather ----------
        for t in range(NT):
            nc.gpsimd.dma_start(h3_in[128 * t:128 * (t + 1), :], h[t][:])
        nc.gpsimd.collective_compute("AllGather", OP.bypass, replica_groups=groups,
                                     ins=[h3_in[:]], outs=[h3_full[:]])

        # ---------- I: patch amax pooling (overlaps ck/cv below) ----------
        pooled = [persist.tile([128, D], bf16, tag=f"pool{pc}", name=f"pool{pc}") for pc in range(NPC)]
        gtmp = tc.alloc_tile_pool(name="gtmp", bufs=3, side="right")
        for pc in range(NPC):
            nc.gpsimd.indirect_dma_start(
                out=pooled[pc][:], out_offset=None, in_=h3_full[:],
                in_offset=bass.IndirectOffsetOnAxis(
                    ap=pgidx_sb[:, 8 * pc:8 * pc + 1], axis=0))
            for s in range(1, 8):
                gt = gtmp.tile([128, D], bf16, tag="gt", name="gt")
                nc.gpsimd.indirect_dma_start(
                    out=gt[:], out_offset=None, in_=h3_full[:],
                    in_offset=bass.IndirectOffsetOnAxis(
                        ap=pgidx_sb[:, 8 * pc + s:8 * pc + s + 1], axis=0))
                nc.vector.tensor_tensor(out=pooled[pc][:], in0=pooled[pc][:],
                                        in1=gt[:], op=OP.max)
            nc.vector.tensor_scalar_mul(pooled[pc][:], pooled[pc][:],
                                        pmask_sb[:, pc:pc + 1])
        gtmp.release()

        hps = tc.alloc_tile_pool(name="hps", bufs=2, space="PSUM", side="right")
        kvn = hn2
        _rms_tiles(nc, small, scratch, h, kvn, NT, eps_sb[:, 0:1])
        kvnT = hn2T
        _transpose_to_dm(nc, tc, kvn, kvnT, ident, NT)

        wcr = tc.alloc_tile_pool(name="wcr", bufs=1, side="right")
        cwk_sb = [wcr.tile([128, D], bf16, tag=f"cwk{c}", name=f"cwk{c}") for c in range(DC)]
        for c in range(DC):
            nc.sync.dma_start(cwk_sb[c][:], w_d["cwk"][128 * c:128 * (c + 1), :])
        ckdm_pool = tc.alloc_tile_pool(name="ckdm_pool", bufs=1, side="right")
        ck_dm = [ckdm_pool.tile([128, T], bf16, tag=f"ckdm{c}", name=f"ckdm{c}") for c in range(DC)]
        for c in range(DC):
            for g in range(2):
                tr = slice(512 * g, 512 * (g + 1))
                ps = hps.tile([128, 512], fp32, tag="hp", name="hp")
                for cc in range(DC):
                    nc.tensor.matmul(ps[:], cwk_sb[cc][:, 128 * c:128 * (c + 1)],
                                     kvnT[cc][:, tr], start=(cc == 0),
                                     stop=(cc == DC - 1))
                nc.vector.tensor_copy(ck_dm[c][:, tr], ps[:])

        cwv_sb = [wcr.tile([128, D], bf16, tag=f"cwv{c}", name=f"cwv{c}") for c in range(DC)]
        for c in range(DC):
            nc.sync.dma_start(cwv_sb[c][:], w_d["cwv"][128 * c:128 * (c + 1), :])
        cv_sb = v_sb
        for t in range(NT):
            ps = hps.tile([128, D], fp32, tag="hp", name="hp")
            for c in range(DC):
                nc.tensor.matmul(ps[:], kvnT[c][:, 128 * t:128 * (t + 1)],
                                 cwv_sb[c][:], start=(c == 0), stop=(c == DC - 1))
            vv = cv_sb[t][:].rearrange("p (h e) -> p h e", h=NH)
            nc.vector.tensor_copy(out=vv[:, :, 0:DH], in_=_head_view(ps[:], h=NH))
            nc.gpsimd.memset(vv[:, :, DH:DH + 1], 1.0)

        for c in range(DC):
            nc.sync.dma_start(ckag_in[128 * c:128 * (c + 1), :], ck_dm[c][:])
        nc.gpsimd.collective_compute("AllGather", OP.bypass, replica_groups=groups,
                                     ins=[ckag_in[:]], outs=[ckag_out[:]])
        for t in range(NT):
            nc.sync.dma_start(cvag_in[128 * t:128 * (t + 1), :], cv_sb[t][:])
        nc.gpsimd.collective_compute("AllGather", OP.bypass, replica_groups=groups,
                                     ins=[cvag_in[:]], outs=[cvag_out[:]])

        hps.release()
        # ---------- J: cross attention ----------
        qn = [persist.tile([128, D], bf16, tag=f"qn{pc}", name=f"qn{pc}") for pc in range(NPC)]
        _rms_tiles(nc, small, scratch, pooled, qn, NPC, eps_sb[:, 0:1])
        qnT = [persist.tile([128, PP], bf16, tag=f"qnT{c}", name=f"qnT{c}") for c in range(DC)]
        _transpose_to_dm(nc, tc, qn, qnT, ident, NPC)

        wcr2 = tc.alloc_tile_pool(name="wcr2", bufs=1, side="right")
        jps = tc.alloc_tile_pool(name="jps", bufs=2, space="PSUM", side="right")
        cwq_sb = [wcr2.tile([128, D], bf16, tag=f"cwq{c}", name=f"cwq{c}") for c in range(DC)]
        for c in range(DC):
            nc.sync.dma_start(cwq_sb[c][:], w_d["cwq"][128 * c:128 * (c + 1), :])
        cq_dm = [persist.tile([128, PP], bf16, tag=f"cqdm{c}", name=f"cqdm{c}") for c in range(DC)]
        for c in range(DC):
            ps = jps.tile([128, PP], fp32, tag="jp", name="jp")
            for cc in range(DC):
                nc.tensor.matmul(ps[:], cwq_sb[cc][:, 128 * c:128 * (c + 1)],
                                 qnT[cc][:], start=(cc == 0), stop=(cc == DC - 1))
            nc.vector.tensor_copy(cq_dm[c][:], ps[:])

        catt_ps = tc.alloc_tile_pool(name="catt_ps", bufs=2, space="PSUM", side="right")
        coT_ps = tc.alloc_tile_pool(name="coT_ps", bufs=2, space="PSUM", side="right")
        cvr_pool = tc.alloc_tile_pool(name="cvr_pool", bufs=1, side="right")
        ckr_pool = tc.alloc_tile_pool(name="ckr_pool", bufs=2, side="right")
        cvr = [cvr_pool.tile([128, VW], bf16, tag=f"cvr{j}", name=f"cvr{j}")
               for j in range(NT)]
        for j in range(NT):
            nc.gpsimd.indirect_dma_start(
                out=cvr[j][:], out_offset=None, in_=cvag_out[:],
                in_offset=bass.IndirectOffsetOnAxis(
                    ap=ridx_sb[:, 4 + j:5 + j], axis=0))
        co_dm = [persist.tile([128, PP], bf16, tag=f"codm{c}", name=f"codm{c}") for c in range(DC)]
        ckr = None
        for hp in range(4):
            c = hp
            ckr = ckr_pool.tile([128, T], bf16, tag="ckr", name="ckr")
            nc.gpsimd.indirect_dma_start(
                out=ckr[:], out_offset=None, in_=ckag_out[:],
                in_offset=bass.IndirectOffsetOnAxis(
                    ap=ridx_sb[:, c:c + 1], axis=0))
            coT = [coT_ps.tile([65, PP], fp32, tag="coT", name="coT")
                   for hf in range(2)]
            for src in range(2):
                ksrc = ck_dm[c] if src == 0 else ckr
                for jj in range(4):
                    for hf in range(2):
                        hh = 2 * hp + hf
                        hrow = 64 * hf
                        cs = catt_ps.tile([128, 512], fp32, tag="cs", name="cs")
                        for u in range(2):
                            j = 2 * jj + u
                            nc.tensor.matmul(
                                cs[:, PP * u:PP * (u + 1)],
                                ksrc[hrow:hrow + 64, 128 * j:128 * (j + 1)],
                                cq_dm[c][hrow:hrow + 64, :], start=True, stop=True)
                        ce = epool.tile([128, 512], bf16, tag="expT", name="expT")
                        nc.scalar.activation(ce[:], cs[:], AF.Exp)
                        for u in range(2):
                            j = 2 * jj + u
                            vt = cv_sb[j] if src == 0 else cvr[j]
                            nc.tensor.matmul(
                                coT[hf][:], vt[:, (DH + 1) * hh:(DH + 1) * hh + DH + 1],
                                ce[:, PP * u:PP * (u + 1)],
                                start=(src == 0 and jj == 0 and u == 0),
                                stop=(src == 1 and jj == 3 and u == 1))
            for hf in range(2):
                hrow = 64 * hf
                rcp = small.tile([1, PP], fp32, tag="crcp", name="crcp")
                nc.scalar.copy(rcp[:], coT[hf][64:65, :])
                rsum = small.tile([1, PP], fp32, tag="crsum", name="crsum")
                nc.vector.reciprocal_approx_fast(rsum[:], rcp[:])
                rbc = scratch.tile([64, PP], fp32, tag="crbc", name="crbc")
                nc.gpsimd.partition_broadcast(rbc[:], rsum[:], channels=64)
                nc.vector.tensor_tensor(out=co_dm[c][hrow:hrow + 64, :],
                                        in0=coT[hf][0:64, :], in1=rbc[:], op=OP.mult)

        cwo_sb = [wcr2.tile([128, D], bf16, tag=f"cwo{c}", name=f"cwo{c}") for c in range(DC)]
        for c in range(DC):
            nc.sync.dma_start(cwo_sb[c][:], w_d["cwo"][128 * c:128 * (c + 1), :])
        for pc in range(NPC):
            ps = jps.tile([128, D], fp32, tag="jp", name="jp")
            for c in range(DC):
                nc.tensor.matmul(ps[:], co_dm[c][:, 128 * pc:128 * (pc + 1)],
                                 cwo_sb[c][:], start=(c == 0), stop=(c == DC - 1))
            ot = scratch.tile([128, D], fp32, tag="outt", name="outt")
            nc.vector.tensor_tensor(out=ot[:], in0=pooled[pc][:], in1=ps[:],
                                    op=OP.add)
            nc.sync.dma_start(out_d[128 * pc:128 * (pc + 1), :], ot[:])

        ckr_pool.release()
        cvr_pool.release()
        coT_ps.release()
        catt_ps.release()
        jps.release()
        wcr2.release()
        ckdm_pool.release()
        wcr.release()
        hn2T_pool.release()
        epool.release()
        scratch.release()
        small.release()
        persist.release()
        const.release()

    nc.compile()
    return nc


_NC_CACHE = None


def kernel(**inputs):
    global _NC_CACHE
    in_maps = prepare_inputs(inputs)
    if _NC_CACHE is None:
        _NC_CACHE = build_nc()
    res = run_bass_kernel_spmd(_NC_CACHE, in_maps, core_ids=list(range(NCORES)))
    return assemble_output(res.results)



# revision 10
# speedup vs baseline: 1.3235x; 1.3235x over previous
"""BLT local encoder on 8 trn2 NeuronCores (Bass/Tile SPMD), v2.

Sharding: 8 cores = 4 batches x 2 parity halves. Core (b, p) embeds the FULL
2048-token sequence locally (batched indirect gathers; no collectives before
attention), computes k/v for the full sequence and q for its own 1024
interleaved-chunk tokens (load-balanced causal), runs attention + FFN on own
tokens, then a single pair AllGather of post-FFN hidden states feeds patch
pooling and full-sequence cross-attention k/v computed locally.

Layouts: residual stream h is token-major bf16 in 16 "swizzled" 128-token
tiles (rows 0:64 = own 64-chunk, 64:128 = peer chunk -> parity-free APs);
q/k are built D-major directly (weights stationary) with RoPE applied in
D-major via a 16-interleaved head permutation + stream_shuffle rotate-half.

Self-contained: shapes hardcoded for
B,S,P,D,NH,DH,F = 4,2048,512,512,8,64,1536, HASH_VOCAB=50002, BYTE_VOCAB=260.
"""
import math
import numpy as np
import ml_dtypes

BF = ml_dtypes.bfloat16

import concourse.bass as bass
import concourse.mybir as mybir
import concourse.tile as tile
from concourse import bacc
from concourse.bass_utils import run_bass_kernel_spmd
from concourse.masks import make_identity

# ----- problem constants (must match reference.py) -----
B, S, P = 4, 2048, 512
D, NH, DH, F = 512, 8, 64, 1536
BYTE_VOCAB = 260
HASH_VOCAB = 50002
GROUP_SIZES = (3, 4)
PRIMES = (1000000007, 5915587277)
N_TABLES = 4
UCAP = 8192           # compacted per-table row capacity (used rows <= B*S)
TBL_ROWS = BYTE_VOCAB + N_TABLES * UCAP
NCORES = 8
T = S // 2            # own tokens per core (1024)
NT = S // 128         # 16 token tiles
NOT = T // 128        # 8 own-token tiles
DC = D // 128         # 4 D chunks
FC = F // 128         # 12 F chunks
PP = P // 2           # own patches per core (256)
NPC = PP // 128       # 2 patch tiles
HALF = DH // 2        # 32
VW = NH * (DH + 1)    # 520
EPS = 1e-5

fp32 = mybir.dt.float32
bf16 = mybir.dt.bfloat16
i32 = mybir.dt.int32
i16 = mybir.dt.int16
AF = mybir.ActivationFunctionType
OP = mybir.AluOpType

SHUF16 = [(i + 16) % 32 for i in range(32)]


# ================= host-side preparation (numpy only) =================

def _hash_indices(tokens):
    Bt, St = tokens.shape
    out = np.zeros((N_TABLES, Bt, St), np.int64)
    idx = 0
    for prime in PRIMES:
        pm = prime % HASH_VOCAB
        for g in GROUP_SIZES:
            xp = np.concatenate([np.zeros((Bt, g - 1), tokens.dtype), tokens], 1)
            hsh = np.zeros((Bt, St), np.int64)
            pw = 1
            for i in range(g):
                hsh = (hsh + xp[:, i:i + St].astype(np.int64) * pw) % HASH_VOCAB
                pw = (pw * pm) % HASH_VOCAB
            out[idx] = hsh
            idx += 1
    return out


def _rope16_perm():
    # within-head dim order: [x1 f0-15, x2 f0-15, x1 f16-31, x2 f16-31]
    o = np.zeros(64, np.int64)
    j = np.arange(16)
    o[0:16] = 2 * j
    o[16:32] = 2 * j + 1
    o[32:48] = 2 * (j + 16)
    o[48:64] = 2 * (j + 16) + 1
    return np.concatenate([64 * h + o for h in range(NH)])


def _rope_tables_dm(positions):
    # D-major rope tables [128, N]: row pattern repeats every 64
    wd = np.arange(64)
    f = np.where(wd < 16, wd,
                 np.where(wd < 32, wd - 16,
                          np.where(wd < 48, wd - 16, wd - 32)))
    sign = np.where(((wd >= 16) & (wd < 32)) | (wd >= 48), 1.0, -1.0)
    theta = 1.0 / (10000.0 ** (f.astype(np.float64) / HALF))
    ang = positions[None, :].astype(np.float64) * theta[:, None]  # [64, N]
    cos = np.cos(ang)
    sin = np.sin(ang) * sign[:, None]
    return (np.tile(cos, (2, 1)).astype(BF), np.tile(sin, (2, 1)).astype(BF))


def _swiz_positions(p):
    i = np.arange(S)
    t, r = i // 128, i % 128
    return np.where(r < 64, 64 * (2 * t + p) + r, 64 * (2 * t + (1 - p)) + r - 64)


def _own_positions(p):
    i = np.arange(T)
    return 64 * (2 * (i // 64) + p) + i % 64


def _attn_masks(p):
    r = np.arange(128)
    c = np.arange(128)
    posq = np.where(r < 64, 64 * p + r, 64 * (2 + p) + (r - 64))
    poskA = np.where(c < 64, 64 * p + c, 64 * (1 - p) + (c - 64))
    poskB = np.where(c < 64, 64 * (2 + p) + c, 64 * (3 - p) + (c - 64))
    # kernel applies masks to e[k_row, q_col] -> ship transposed
    maskA = (posq[None, :] >= poskA[:, None]).astype(np.float32)
    maskB = (posq[None, :] >= poskB[:, None]).astype(np.float32)
    return maskA.astype(BF), maskB.astype(BF)


def _ag_pos(g):
    ch = g // 64
    return 1024 * (ch % 2) + 64 * (ch // 2) + g % 64


def _pool_indices(pl_b, p):
    cum = np.cumsum(pl_b)
    starts = np.concatenate([[0], cum[:-1]])
    ends = cum
    pgidx = np.zeros((128, 16), np.int32)
    pmask = np.zeros((128, 2), np.float32)
    for pc in range(NPC):
        for r in range(128):
            patch = 256 * p + 128 * pc + r
            st, en = int(starts[patch]), int(min(ends[patch], S))
            if st >= S or en <= st:
                pmask[r, pc] = 0.0
            else:
                sl = np.minimum(st + np.arange(8), en - 1)
                pgidx[r, 8 * pc:8 * pc + 8] = [_ag_pos(int(x)) for x in sl]
                pmask[r, pc] = 1.0
    return pgidx, pmask


def prepare_inputs(inputs):
    tokens = np.asarray(inputs["tokens"])
    pl = np.asarray(inputs["patch_lengths"])
    tok_emb = np.asarray(inputs["tok_emb"], np.float32)
    hash_emb = np.asarray(inputs["hash_emb"], np.float32)

    hidx = _hash_indices(tokens)          # [4, B, S] raw values < HASH_VOCAB
    parts = [tok_emb.astype(BF)]
    hinv = np.zeros_like(hidx, dtype=np.int64)
    for s_ in range(N_TABLES):
        used = np.unique(hidx[s_])
        assert used.size <= UCAP
        tbl = np.zeros((UCAP, D), BF)
        tbl[:used.size] = hash_emb[s_][used].astype(BF)
        parts.append(tbl)
        hinv[s_] = np.searchsorted(used, hidx[s_])
    tables = np.ascontiguousarray(np.concatenate(parts, 0))
    assert tables.shape == (TBL_ROWS, D)

    perm = _rope16_perm()
    wq = np.ascontiguousarray(
        np.asarray(inputs["wq"], np.float32)[:, perm] * (1.0 / math.sqrt(DH)))
    wk = np.ascontiguousarray(np.asarray(inputs["wk"], np.float32)[:, perm])
    cwq = np.ascontiguousarray(
        np.asarray(inputs["cwq"], np.float32) * (1.0 / math.sqrt(DH)))

    shared = {
        "tables": tables, "wq": wq.astype(BF), "wk": wk.astype(BF),
        "wv": np.asarray(inputs["wv"], np.float32).astype(BF),
        "wo": np.asarray(inputs["wo"], np.float32).astype(BF),
        "w1": np.asarray(inputs["w1"], np.float32).astype(BF),
        "w3": np.asarray(inputs["w3"], np.float32).astype(BF),
        "w2": np.asarray(inputs["w2"], np.float32).astype(BF),
        "cwq": cwq.astype(BF),
        "cwk": np.asarray(inputs["cwk"], np.float32).astype(BF),
        "cwv": np.asarray(inputs["cwv"], np.float32).astype(BF),
        "cwo": np.asarray(inputs["cwo"], np.float32).astype(BF),
    }

    in_maps = []
    for core in range(NCORES):
        b, p = core // 2, core % 2
        swiz = _swiz_positions(p)
        # embedding gather indices: per (chunk of 4 tiles, table) a [16, 32]
        # int16 block, wrapped and replicated to all 128 partitions
        planes = [tokens[b, swiz].astype(np.int64)] + \
                 [hinv[s_, b, swiz] for s_ in range(N_TABLES)]
        ew = np.zeros((16, 32 * 4 * 5), np.int16)
        i2 = np.arange(512)
        for ch in range(4):
            for tb in range(5):
                ew[i2 % 16, 32 * (5 * ch + tb) + i2 // 16] = \
                    planes[tb][512 * ch + i2]
        embidx = np.tile(ew, (8, 1))
        own = _own_positions(p)
        cosK, sinK = _rope_tables_dm(swiz)
        cosQ, sinQ = _rope_tables_dm(own)
        maskA, maskB = _attn_masks(p)
        pgidx, pmask = _pool_indices(pl[b], p)
        # pooling gathers as dma_gather idx lists: idx[j*128+r] = pgidx[r, 8pc+j]
        pw = np.zeros((16, 128), np.int16)
        for pc in range(NPC):
            i3 = np.arange(1024)
            pw[i3 % 16, 64 * pc + i3 // 16] = pgidx[i3 % 128, 8 * pc + i3 // 128]
        pgidx16 = np.tile(pw, (8, 1))
        # peer h3 rows: idx[j*128+r] = (1-p)*1024 + 128j + r
        i3 = np.arange(1024)
        rw = np.zeros((16, 64), np.int16)
        rw[i3 % 16, i3 // 16] = (1 - p) * T + i3
        ridx16 = np.tile(rw, (8, 1))
        m = dict(shared)
        m.update({"embidx": embidx,
                  "cosk": cosK, "sink": sinK, "cosq": cosQ, "sinq": sinQ,
                  "maska": maskA, "maskb": maskB,
                  "pgidx16": pgidx16, "pmask": pmask, "ridx16": ridx16})
        in_maps.append(m)
    return in_maps


def assemble_output(results):
    out = np.zeros((B, P, D), np.float32)
    for core in range(NCORES):
        b, p = core // 2, core % 2
        out[b, 256 * p:256 * (p + 1), :] = results[core]["out"]
    return out


# ========================= kernel build =========================

def build_nc():
    nc = bacc.Bacc("TRN2", target_bir_lowering=False, debug=False,
                   num_devices=NCORES, num_swdge_queues=4)

    # ---- I/O ----
    tables = nc.dram_tensor("tables", [TBL_ROWS, D], bf16, kind="ExternalInput").ap()
    embidx_d = nc.dram_tensor("embidx", [128, 32 * 4 * 5], i16, kind="ExternalInput").ap()
    w_d = {}
    for nm in ("wq", "wk", "wv", "wo", "cwq", "cwk", "cwv", "cwo"):
        w_d[nm] = nc.dram_tensor(nm, [D, D], bf16, kind="ExternalInput").ap()
    w1_d = nc.dram_tensor("w1", [D, F], bf16, kind="ExternalInput").ap()
    w3_d = nc.dram_tensor("w3", [D, F], bf16, kind="ExternalInput").ap()
    w2_d = nc.dram_tensor("w2", [F, D], bf16, kind="ExternalInput").ap()
    cosk_d = nc.dram_tensor("cosk", [128, S], bf16, kind="ExternalInput").ap()
    sink_d = nc.dram_tensor("sink", [128, S], bf16, kind="ExternalInput").ap()
    cosq_d = nc.dram_tensor("cosq", [128, T], bf16, kind="ExternalInput").ap()
    sinq_d = nc.dram_tensor("sinq", [128, T], bf16, kind="ExternalInput").ap()
    maska_d = nc.dram_tensor("maska", [128, 128], bf16, kind="ExternalInput").ap()
    maskb_d = nc.dram_tensor("maskb", [128, 128], bf16, kind="ExternalInput").ap()
    pgidx_d = nc.dram_tensor("pgidx16", [128, 128], i16, kind="ExternalInput").ap()
    pmask_d = nc.dram_tensor("pmask", [128, 2], fp32, kind="ExternalInput").ap()
    ridx_d = nc.dram_tensor("ridx16", [128, 64], i16, kind="ExternalInput").ap()
    out_d = nc.dram_tensor("out", [PP, D], fp32, kind="ExternalOutput").ap()

    # ---- internal DRAM ----
    h3_in = nc.dram_tensor("h3_in", [T, D], bf16).ap()
    h3_full = nc.dram_tensor("h3_full", [S, D], bf16).ap()

    groups = [[2 * i, 2 * i + 1] for i in range(NCORES // 2)]

    with tile.TileContext(nc) as tc:
        const = tc.alloc_tile_pool(name="const", bufs=1, side="left")
        hpool = tc.alloc_tile_pool(name="hpool", bufs=1, side="left")
        hnpool = tc.alloc_tile_pool(name="hnpool", bufs=1, side="left")
        small = tc.alloc_tile_pool(name="small", bufs=2, side="left")
        scratch = tc.alloc_tile_pool(name="scratch", bufs=2, side="left")

        # long-lived right-side pools (alloc first; freed implicitly at end)
        ownT = tc.alloc_tile_pool(name="ownT", bufs=1, side="right")
        kpool = tc.alloc_tile_pool(name="kpool", bufs=1, side="right")
        qpool = tc.alloc_tile_pool(name="qpool", bufs=1, side="right")
        vpool = tc.alloc_tile_pool(name="vpool", bufs=1, side="right")
        opool = tc.alloc_tile_pool(name="opool", bufs=1, side="right")
        epool = tc.alloc_tile_pool(name="epool", bufs=3, side="right")
        rtmp = tc.alloc_tile_pool(name="rtmp", bufs=2, side="right")

        # ---------- constants ----------
        ident = const.tile([128, 128], bf16)
        make_identity(nc, ident[:])
        embidx = const.tile([128, 32 * 4 * 5], i16)
        nc.sync.dma_start(embidx[:], embidx_d[:])
        cosk = const.tile([128, S], bf16)
        nc.sync.dma_start(cosk[:], cosk_d[:])
        sink = const.tile([128, S], bf16)
        nc.sync.dma_start(sink[:], sink_d[:])
        cosq = const.tile([128, T], bf16)
        nc.sync.dma_start(cosq[:], cosq_d[:])
        sinq = const.tile([128, T], bf16)
        nc.sync.dma_start(sinq[:], sinq_d[:])
        maska = const.tile([128, 128], bf16)
        nc.sync.dma_start(maska[:], maska_d[:])
        maskb = const.tile([128, 128], bf16)
        nc.sync.dma_start(maskb[:], maskb_d[:])
        pgidx_sb = const.tile([128, 128], i16)
        nc.sync.dma_start(pgidx_sb[:], pgidx_d[:])
        pmask_sb = const.tile([128, 2], fp32)
        nc.sync.dma_start(pmask_sb[:], pmask_d[:])
        ridx_sb = const.tile([128, 64], i16)
        nc.sync.dma_start(ridx_sb[:], ridx_d[:])
        eps_sb = const.tile([128, 1], fp32)
        nc.gpsimd.memset(eps_sb[:], EPS)

        # residual stream: one big bf16 tile, 16 swizzled token-tile views
        hbig = hpool.tile([128, NT * D], bf16, name="hbig")
        h = [hbig[:, D * t:D * (t + 1)] for t in range(NT)]

        def hn_chunk(nm):
            # scratch for 4 consecutive rms'd tiles (consumed by transposes)
            big = hnpool.tile([128, 4 * D], bf16, tag="hnc", name=nm, bufs=2)
            return big, [big[:, D * k:D * (k + 1)] for k in range(4)]

        # ---------- helpers ----------
        def rms_tile(dst, src):
            ssq = small.tile([128, 1], fp32, tag="ssq", name="ssq")
            nc.vector.scalar_tensor_tensor(
                out=dst, in0=src, scalar=1.0, in1=src,
                op0=OP.mult, op1=OP.mult, accum_out=ssq[:])
            sq = small.tile([128, 1], fp32, tag="sqm", name="sqm")
            nc.scalar.activation(sq[:], ssq[:], AF.Sqrt, bias=eps_sb[:, 0:1],
                                 scale=1.0 / D)
            rs = small.tile([128, 1], fp32, tag="rs", name="rs")
            nc.vector.reciprocal_approx_fast(rs[:], sq[:])
            nc.vector.tensor_scalar_mul(dst, src, rs[:, 0:1])

        def rope_apply(dst, ps, cosT, sinT):
            kraw = rtmp.tile([128, 512], bf16, tag="kraw", name="kraw")
            nc.scalar.copy(kraw[:], ps)
            ksw = rtmp.tile([128, 512], bf16, tag="ksw", name="ksw")
            nc.vector.stream_shuffle(ksw[:], kraw[:], SHUF16)
            nc.vector.tensor_tensor(out=ksw[:], in0=ksw[:], in1=sinT, op=OP.mult)
            t1 = rtmp.tile([128, 512], bf16, tag="rt1", name="rt1")
            nc.vector.tensor_tensor(out=t1[:], in0=kraw[:], in1=cosT, op=OP.mult)
            nc.vector.tensor_tensor(out=dst, in0=t1[:], in1=ksw[:], op=OP.add)

        # tensors
        hnO = [ownT.tile([128, T], bf16, tag=f"ownT{c}", name=f"hnO{c}")
               for c in range(DC)]
        k_dm = [kpool.tile([128, S], bf16, tag=f"kdm{c}", name=f"kdm{c}")
                for c in range(DC)]
        q_dm = [qpool.tile([128, T], bf16, tag=f"qdm{c}", name=f"qdm{c}")
                for c in range(DC)]
        v_sb = [vpool.tile([128, VW], bf16, tag=f"v{t}", name=f"v{t}")
                for t in range(NT)]
        o_dm = [opool.tile([128, T], bf16, tag=f"odm{c}", name=f"odm{c}")
                for c in range(DC)]

        # transient pools for phases A-C (LIFO release order)
        dmT = tc.alloc_tile_pool(name="dmT", bufs=1, side="right")
        wqkv = tc.alloc_tile_pool(name="wqkv", bufs=1, side="right")
        ghp = tc.alloc_tile_pool(name="ghp", bufs=2, side="right")


        wk_sb = [wqkv.tile([128, D], bf16, tag=f"wk{c}", name=f"wk{c}") for c in range(DC)]
        wq_sb = [wqkv.tile([128, D], bf16, tag=f"wq{c}", name=f"wq{c}") for c in range(DC)]
        wv_sb = [wqkv.tile([128, D], bf16, tag=f"wv{c}", name=f"wv{c}") for c in range(DC)]
        for c in range(DC):
            nc.sync.dma_start(wk_sb[c][:], w_d["wk"][128 * c:128 * (c + 1), :])
            nc.sync.dma_start(wq_sb[c][:], w_d["wq"][128 * c:128 * (c + 1), :])
            nc.sync.dma_start(wv_sb[c][:], w_d["wv"][128 * c:128 * (c + 1), :])

        trans_ps = tc.alloc_tile_pool(name="trans_ps", bufs=2, space="PSUM",
                                      side="right")
        proj_ps = tc.alloc_tile_pool(name="proj_ps", bufs=3, space="PSUM",
                                     side="right")

        # ---------- A+B+C fused: embed -> rms -> transpose -> k/v ----------
        # per 4-tile chunk: 5 dma_gathers (tok + 4 compacted hash tables)
        for tq in range(4):
            gt = []
            for tb in range(5):
                g_ = ghp.tile([128, 4, D], bf16, tag=f"g{tb}", name=f"g{tb}")
                if tb == 0:
                    src = tables[0:BYTE_VOCAB, :]
                else:
                    src = tables[BYTE_VOCAB + (tb - 1) * UCAP:
                                 BYTE_VOCAB + tb * UCAP, :]
                nc.gpsimd.dma_gather(
                    out_ap=g_[:], in_ap=src,
                    idxs_ap=embidx[:, 32 * (5 * tq + tb):32 * (5 * tq + tb + 1)],
                    num_idxs=512, num_idxs_reg=512, elem_size=D,
                    queue_num=tb % 4)
                gt.append(g_)
            _, hnc = hn_chunk(f"hn{tq}")
            for kk in range(4):
                t = 4 * tq + kk
                a = ghp.tile([128, D], bf16, tag="ga", name="ga", bufs=1)
                bb = ghp.tile([128, D], bf16, tag="gb", name="gb", bufs=1)
                nc.vector.tensor_tensor(out=a[:], in0=gt[0][:, kk, :],
                                        in1=gt[1][:, kk, :], op=OP.add)
                nc.vector.tensor_tensor(out=bb[:], in0=gt[2][:, kk, :],
                                        in1=gt[3][:, kk, :], op=OP.add)
                nc.vector.tensor_tensor(out=a[:], in0=a[:], in1=bb[:], op=OP.add)
                nc.vector.tensor_tensor(out=h[t], in0=a[:], in1=gt[4][:, kk, :],
                                        op=OP.add)
                rms_tile(hnc[kk], h[t])
            hnT = [dmT.tile([128, 512], bf16, tag=f"dmT{c}", name=f"hnT{c}")
                   for c in range(DC)]
            for c in range(DC):
                tp = trans_ps.tile([128, 512], bf16, tag="tp", name="tp")
                for kk in range(4):
                    nc.tensor.transpose(out=tp[:, 128 * kk:128 * (kk + 1)],
                                        in_=hnc[kk][:, 128 * c:128 * (c + 1)],
                                        identity=ident[:])
                nc.scalar.copy(hnT[c][:], tp[:])
            # own-column transposes for this chunk
            for c in range(DC):
                tp = trans_ps.tile([128, 512], bf16, tag="tp", name="tp")
                for kk in range(4):
                    nc.tensor.transpose(
                        out=tp[:, 64 * kk:64 * (kk + 1)],
                        in_=hnc[kk][0:64, 128 * c:128 * (c + 1)],
                        identity=ident[0:64, 0:64])
                nc.scalar.copy(hnO[c][:, 256 * tq:256 * (tq + 1)], tp[:, 0:256])
            # k projection for this 512-token group
            for co in range(DC):
                pk = proj_ps.tile([128, 512], fp32, tag="pj", name="pk")
                for ci in range(DC):
                    nc.tensor.matmul(pk[:], wk_sb[ci][:, 128 * co:128 * (co + 1)],
                                     hnT[ci][:], start=(ci == 0), stop=(ci == DC - 1))
                rope_apply(k_dm[co][:, 512 * tq:512 * (tq + 1)], pk[:],
                           cosk[:, 512 * tq:512 * (tq + 1)],
                           sink[:, 512 * tq:512 * (tq + 1)])
            # v for this chunk (token-major, ones column)
            for kk in range(4):
                t = 4 * tq + kk
                pv = proj_ps.tile([128, 512], fp32, tag="pj", name="pv")
                for ci in range(DC):
                    nc.tensor.matmul(pv[:], hnT[ci][:, 128 * kk:128 * (kk + 1)],
                                     wv_sb[ci][:], start=(ci == 0),
                                     stop=(ci == DC - 1))
                vv = v_sb[t][:].rearrange("p (h e) -> p h e", h=NH)
                nc.scalar.copy(vv[:, :, 0:DH],
                               pv[:].rearrange("p (h e) -> p h e", h=NH))
                nc.gpsimd.memset(vv[:, :, DH:DH + 1], 1.0)

        for g in range(2):
            for co in range(DC):
                pq = proj_ps.tile([128, 512], fp32, tag="pj", name="pq")
                for ci in range(DC):
                    nc.tensor.matmul(pq[:], wq_sb[ci][:, 128 * co:128 * (co + 1)],
                                     hnO[ci][:, 512 * g:512 * (g + 1)],
                                     start=(ci == 0), stop=(ci == DC - 1))
                rope_apply(q_dm[co][:, 512 * g:512 * (g + 1)], pq[:],
                           cosq[:, 512 * g:512 * (g + 1)],
                           sinq[:, 512 * g:512 * (g + 1)])
        proj_ps.release()
        trans_ps.release()
        ghp.release()
        wqkv.release()
        dmT.release()

        # ---------- D: self attention ----------
        sc_ps = tc.alloc_tile_pool(name="sc_ps", bufs=2, space="PSUM", side="right")
        oT_ps = tc.alloc_tile_pool(name="oT_ps", bufs=4, space="PSUM", side="right")

        for hp in range(4):
            for g in range(2):
                for hf in range(2):
                    hrow = 64 * hf
                    hh = 2 * hp + hf
                    oT = oT_ps.tile([65, 512], fp32, tag="oT", name="oT")
                    umax = 4 * g + 3
                    pend = None  # (e, odd_off, N, s0, u) awaiting AV
                    for u in range(umax + 1):
                        s0 = 128 * max(0, u - 4 * g)
                        N = 512 - s0
                        odd_off = 512 if N >= 384 else N
                        sc = sc_ps.tile([128, 1024], fp32, tag="sc", name="sc")
                        for w in range(2):
                            j = 2 * u + w
                            nc.tensor.matmul(
                                sc[:, odd_off * w:odd_off * w + N],
                                k_dm[hp][hrow:hrow + 64, 128 * j:128 * (j + 1)],
                                q_dm[hp][hrow:hrow + 64,
                                         512 * g + s0:512 * (g + 1)],
                                start=True, stop=True)
                        e = epool.tile([128, 1024], bf16, tag="e", name="e")
                        if odd_off == 512 and N == 512:
                            nc.scalar.activation(e[:], sc[:], AF.Exp)
                        elif odd_off == 512:
                            nc.scalar.activation(e[:, 0:N], sc[:, 0:N], AF.Exp)
                            nc.scalar.activation(e[:, 512:512 + N],
                                                 sc[:, 512:512 + N], AF.Exp)
                        else:
                            nc.scalar.activation(e[:, 0:2 * N], sc[:, 0:2 * N],
                                                 AF.Exp)
                        if u >= 4 * g:
                            nc.vector.tensor_tensor(
                                out=e[:, 0:128], in0=e[:, 0:128], in1=maska[:],
                                op=OP.mult)
                            nc.vector.tensor_tensor(
                                out=e[:, odd_off:odd_off + 128],
                                in0=e[:, odd_off:odd_off + 128], in1=maskb[:],
                                op=OP.mult)

                        def av(item, last):
                            e_, oo_, N_, s0_, u_ = item
                            for w in range(2):
                                j = 2 * u_ + w
                                nc.tensor.matmul(
                                    oT[:, s0_:512],
                                    v_sb[j][:, 65 * hh:65 * hh + 65],
                                    e_[:, oo_ * w:oo_ * w + N_],
                                    start=(u_ == 0 and w == 0),
                                    stop=(last and w == 1))
                        if pend is not None:
                            av(pend, False)
                        pend = (e, odd_off, N, s0, u)
                    av(pend, True)
                    rcp = small.tile([1, 512], fp32, tag="rcp", name="rcp")
                    nc.scalar.copy(rcp[:], oT[64:65, :])
                    rsum = small.tile([1, 512], fp32, tag="rsum", name="rsum")
                    nc.vector.reciprocal_approx_fast(rsum[:], rcp[:])
                    rbc = scratch.tile([64, 512], fp32, tag="rbc", name="rbc")
                    nc.gpsimd.partition_broadcast(rbc[:], rsum[:], channels=64)
                    nc.vector.tensor_tensor(
                        out=o_dm[hp][hrow:hrow + 64, 512 * g:512 * (g + 1)],
                        in0=oT[0:64, :], in1=rbc[:], op=OP.mult)
        oT_ps.release()
        sc_ps.release()

        # ---------- E: out-proj + residual ----------
        proj2 = tc.alloc_tile_pool(name="proj2", bufs=3, space="PSUM", side="right")
        trans2 = tc.alloc_tile_pool(name="trans2", bufs=2, space="PSUM", side="right")
        wos = tc.alloc_tile_pool(name="wos", bufs=1, side="right")
        wo_sb = [wos.tile([128, D], bf16, tag=f"wo{c}", name=f"wo{c}") for c in range(DC)]
        for c in range(DC):
            nc.sync.dma_start(wo_sb[c][:], w_d["wo"][128 * c:128 * (c + 1), :])

        def resid_add(gt, ps):
            nc.vector.tensor_tensor(
                out=h[2 * gt][0:64, :], in0=h[2 * gt][0:64, :],
                in1=ps[0:64, :], op=OP.add)
            nc.vector.tensor_tensor(
                out=h[2 * gt + 1][0:64, :], in0=h[2 * gt + 1][0:64, :],
                in1=ps[64:128, :], op=OP.add)

        for gt in range(NOT):
            po = proj2.tile([128, D], fp32, tag="pj2", name="po")
            for c in range(DC):
                nc.tensor.matmul(po[:], o_dm[c][:, 128 * gt:128 * (gt + 1)],
                                 wo_sb[c][:], start=(c == 0), stop=(c == DC - 1))
            resid_add(gt, po[:])
        wos.release()

        # ---------- F: rms2 + hn2T-own ----------
        hn2O = [ownT.tile([128, T], bf16, tag=f"ownT{c}", name=f"hn2O{c}")
                for c in range(DC)]
        for tq in range(4):
            _, hnc = hn_chunk(f"hn2_{tq}")
            for kk in range(4):
                rms_tile(hnc[kk], h[4 * tq + kk])
            for c in range(DC):
                tp = trans2.tile([128, 512], bf16, tag="tp2", name="tp2")
                for kk in range(4):
                    nc.tensor.transpose(
                        out=tp[:, 64 * kk:64 * (kk + 1)],
                        in_=hnc[kk][0:64, 128 * c:128 * (c + 1)],
                        identity=ident[0:64, 0:64])
                nc.scalar.copy(hn2O[c][:, 256 * tq:256 * (tq + 1)], tp[:, 0:256])

        # ---------- G: FFN ----------
        wf = tc.alloc_tile_pool(name="wf", bufs=1, side="right")
        mid_pool = tc.alloc_tile_pool(name="mid_pool", bufs=1, side="right")
        mid = [mid_pool.tile([128, T], bf16, tag=f"mid{f}", name=f"mid{f}")
               for f in range(FC)]
        w1_sb = [wf.tile([128, F], bf16, tag=f"w1s{c}", name=f"w1s{c}") for c in range(DC)]
        w3_sb = [wf.tile([128, F], bf16, tag=f"w3s{c}", name=f"w3s{c}") for c in range(DC)]
        for c in range(DC):
            nc.sync.dma_start(w1_sb[c][:], w1_d[128 * c:128 * (c + 1), :])
            nc.sync.dma_start(w3_sb[c][:], w3_d[128 * c:128 * (c + 1), :])
        for f in range(FC):
            fs = slice(128 * f, 128 * (f + 1))
            for g in range(2):
                tr = slice(512 * g, 512 * (g + 1))
                gp = proj2.tile([128, 512], fp32, tag="pj2", name="gp")
                up = proj2.tile([128, 512], fp32, tag="pj2", name="up")
                for c in range(DC):
                    nc.tensor.matmul(gp[:], w1_sb[c][:, fs], hn2O[c][:, tr],
                                     start=(c == 0), stop=(c == DC - 1))
                for c in range(DC):
                    nc.tensor.matmul(up[:], w3_sb[c][:, fs], hn2O[c][:, tr],
                                     start=(c == 0), stop=(c == DC - 1))
                gs = scratch.tile([128, 512], bf16, tag="gs", name="gs")
                nc.scalar.activation(gs[:], gp[:], AF.Silu)
                nc.vector.tensor_tensor(out=mid[f][:, tr], in0=gs[:], in1=up[:],
                                        op=OP.mult)
        w2_sb = [wf.tile([128, D], bf16, tag=f"w2s{f}", name=f"w2s{f}") for f in range(FC)]
        for f in range(FC):
            nc.sync.dma_start(w2_sb[f][:], w2_d[128 * f:128 * (f + 1), :])
        for gt in range(NOT):
            pd = proj2.tile([128, D], fp32, tag="pj2", name="pd")
            for f in range(FC):
                nc.tensor.matmul(pd[:], mid[f][:, 128 * gt:128 * (gt + 1)],
                                 w2_sb[f][:], start=(f == 0), stop=(f == FC - 1))
            resid_add(gt, pd[:])
        mid_pool.release()
        wf.release()

        # ---------- H: ship h3, AllGather, cross-kv ----------
        # one DMA: own rows (0:64 of each tile) -> h3_in [1024, 512]
        h3_src = hbig[0:64, :].rearrange("r (t d) -> r t d", d=D)
        h3_dst = h3_in[:].rearrange("(t r) d -> r t d", r=64)
        nc.sync.dma_start(h3_dst, h3_src)
        nc.gpsimd.collective_compute(
            "AllGather", OP.bypass, replica_groups=groups,
            ins=[h3_in[:]], outs=[h3_full[:]])

        # kvn (own rows valid) + kvnT-own + ck/cv own — overlaps AllGather
        kvnO = [ownT.tile([128, T], bf16, tag=f"ownT{c}", name=f"kvnO{c}")
                for c in range(DC)]
        for tq in range(4):
            _, hnc = hn_chunk(f"kvn{tq}")
            for kk in range(4):
                rms_tile(hnc[kk], h[4 * tq + kk])
            for c in range(DC):
                tp = trans2.tile([128, 512], bf16, tag="tp2", name="tp2")
                for kk in range(4):
                    nc.tensor.transpose(
                        out=tp[:, 64 * kk:64 * (kk + 1)],
                        in_=hnc[kk][0:64, 128 * c:128 * (c + 1)],
                        identity=ident[0:64, 0:64])
                nc.scalar.copy(kvnO[c][:, 256 * tq:256 * (tq + 1)], tp[:, 0:256])

        wcr = tc.alloc_tile_pool(name="wcr", bufs=1, side="right")
        cwk_sb = [wcr.tile([128, D], bf16, tag=f"cwk{c}", name=f"cwk{c}") for c in range(DC)]
        cwv_sb = [wcr.tile([128, D], bf16, tag=f"cwv{c}", name=f"cwv{c}") for c in range(DC)]
        cwq_sb = [wcr.tile([128, D], bf16, tag=f"cwq{c}", name=f"cwq{c}") for c in range(DC)]
        cwo_sb = [wcr.tile([128, D], bf16, tag=f"cwo{c}", name=f"cwo{c}") for c in range(DC)]
        for c in range(DC):
            nc.sync.dma_start(cwk_sb[c][:], w_d["cwk"][128 * c:128 * (c + 1), :])
            nc.sync.dma_start(cwv_sb[c][:], w_d["cwv"][128 * c:128 * (c + 1), :])
            nc.sync.dma_start(cwq_sb[c][:], w_d["cwq"][128 * c:128 * (c + 1), :])
            nc.sync.dma_start(cwo_sb[c][:], w_d["cwo"][128 * c:128 * (c + 1), :])

        ck_dm = [kpool.tile([128, S], bf16, tag=f"kdm{c}", name=f"ckdm{c}")
                 for c in range(DC)]
        cv_sb = [vpool.tile([128, VW], bf16, tag=f"v{t}", name=f"cv{t}")
                 for t in range(NT)]

        def ck_group(co, dst_cols, rhs):
            pc = proj2.tile([128, 512], fp32, tag="pj2", name="pck")
            for ci in range(DC):
                nc.tensor.matmul(pc[:], cwk_sb[ci][:, 128 * co:128 * (co + 1)],
                                 rhs[ci], start=(ci == 0), stop=(ci == DC - 1))
            nc.scalar.copy(ck_dm[co][:, dst_cols], pc[:])

        def cv_tile(t, lhs):
            pv = proj2.tile([128, 512], fp32, tag="pj2", name="pcv")
            for ci in range(DC):
                nc.tensor.matmul(pv[:], lhs[ci], cwv_sb[ci][:],
                                 start=(ci == 0), stop=(ci == DC - 1))
            vv = cv_sb[t][:].rearrange("p (h e) -> p h e", h=NH)
            nc.scalar.copy(vv[:, :, 0:DH],
                           pv[:].rearrange("p (h e) -> p h e", h=NH))
            nc.gpsimd.memset(vv[:, :, DH:DH + 1], 1.0)

        for g in range(2):
            cols = slice(512 * g, 512 * (g + 1))
            for co in range(DC):
                ck_group(co, cols, [kvnO[ci][:, cols] for ci in range(DC)])
        for t in range(NOT):
            cv_tile(t, [kvnO[ci][:, 128 * t:128 * (t + 1)] for ci in range(DC)])

        # ---- after AllGather: peer rows + pooling ----
        h3rp = tc.alloc_tile_pool(name="h3rp", bufs=1, side="right")
        kvnrp = tc.alloc_tile_pool(name="kvnrp", bufs=1, side="right")
        plp = tc.alloc_tile_pool(name="plp", bufs=2, side="right")

        h3r = h3rp.tile([128, NOT * D], bf16, name="h3rbig")
        nc.gpsimd.dma_gather(
            out_ap=h3r[:].rearrange("p (t d) -> p t d", d=D),
            in_ap=h3_full[:], idxs_ap=ridx_sb[:],
            num_idxs=T, num_idxs_reg=T, elem_size=D, queue_num=1)
        kvnr = kvnrp.tile([128, NOT * D], bf16, name="kvnrbig")
        for t in range(NOT):
            rms_tile(kvnr[:, D * t:D * (t + 1)], h3r[:, D * t:D * (t + 1)])
        kvnrT = [qpool.tile([128, T], bf16, tag=f"qdm{c}", name=f"kvnrT{c}")
                 for c in range(DC)]
        for c in range(DC):
            for half in range(2):
                tp = trans2.tile([128, 512], bf16, tag="tp2", name="tp2")
                for kk in range(4):
                    t = 4 * half + kk
                    nc.tensor.transpose(
                        out=tp[:, 128 * kk:128 * (kk + 1)],
                        in_=kvnr[:, D * t + 128 * c:D * t + 128 * (c + 1)],
                        identity=ident[:])
                nc.scalar.copy(kvnrT[c][:, 512 * half:512 * (half + 1)], tp[:])
        for g in range(2):
            for co in range(DC):
                ck_group(co, slice(T + 512 * g, T + 512 * (g + 1)),
                         [kvnrT[ci][:, 512 * g:512 * (g + 1)] for ci in range(DC)])
        for t in range(NOT):
            cv_tile(NOT + t,
                    [kvnrT[ci][:, 128 * t:128 * (t + 1)] for ci in range(DC)])

        # pooling (amax over patch bytes)
        pooled = [hpool.tile([128, D], bf16, tag=f"pool{pc}", name=f"pool{pc}")
                  for pc in range(NPC)]
        for pc in range(NPC):
            pg = plp.tile([128, 8 * D], bf16, tag="pg", name="pg")
            nc.gpsimd.dma_gather(
                out_ap=pg[:].rearrange("p (t d) -> p t d", d=D),
                in_ap=h3_full[:], idxs_ap=pgidx_sb[:, 64 * pc:64 * (pc + 1)],
                num_idxs=T, num_idxs_reg=T, elem_size=D, queue_num=2)
            nc.vector.tensor_tensor(out=pooled[pc][:], in0=pg[:, 0:D],
                                    in1=pg[:, D:2 * D], op=OP.max)
            for s_ in range(2, 8):
                nc.vector.tensor_tensor(out=pooled[pc][:], in0=pooled[pc][:],
                                        in1=pg[:, D * s_:D * (s_ + 1)], op=OP.max)
            nc.vector.tensor_scalar_mul(pooled[pc][:], pooled[pc][:],
                                        pmask_sb[:, pc:pc + 1])

        # qn + qnT + cq
        qn = [hpool.tile([128, D], bf16, tag=f"qn{pc}", name=f"qn{pc}")
              for pc in range(NPC)]
        for pc in range(NPC):
            rms_tile(qn[pc][:], pooled[pc][:])
        qnT = [scratch.tile([128, PP], bf16, tag=f"qnT{c}", name=f"qnT{c}", bufs=1)
               for c in range(DC)]
        for c in range(DC):
            tp = trans2.tile([128, 512], bf16, tag="tp2", name="tp2")
            for pc in range(NPC):
                nc.tensor.transpose(out=tp[:, 128 * pc:128 * (pc + 1)],
                                    in_=qn[pc][:, 128 * c:128 * (c + 1)],
                                    identity=ident[:])
            nc.scalar.copy(qnT[c][:], tp[:, 0:PP])
        cq_dm = [scratch.tile([128, PP], bf16, tag=f"cq{c}", name=f"cq{c}", bufs=1)
                 for c in range(DC)]
        for co in range(DC):
            pq = proj2.tile([128, 512], fp32, tag="pj2", name="pcq")
            for ci in range(DC):
                nc.tensor.matmul(pq[:, 0:PP],
                                 cwq_sb[ci][:, 128 * co:128 * (co + 1)],
                                 qnT[ci][:], start=(ci == 0), stop=(ci == DC - 1))
            nc.scalar.copy(cq_dm[co][:], pq[:, 0:PP])
        trans2.release()
        proj2.release()

        # ---------- J: cross attention ----------
        proj3 = tc.alloc_tile_pool(name="proj3", bufs=2, space="PSUM", side="right")
        cs_ps = tc.alloc_tile_pool(name="cs_ps", bufs=2, space="PSUM", side="right")
        coT_ps = tc.alloc_tile_pool(name="coT_ps", bufs=2, space="PSUM", side="right")
        co_dm = [opool.tile([128, T], bf16, tag=f"odm{c}", name=f"codm{c}")
                 for c in range(DC)]
        for hp in range(4):
            for hf in range(2):
                hrow = 64 * hf
                hh = 2 * hp + hf
                coT = coT_ps.tile([65, PP], fp32, tag="coT", name="coT")
                pend = None
                for q4 in range(4):
                    cs = cs_ps.tile([128, 1024], fp32, tag="cs", name="cs")
                    for w in range(4):
                        j = 4 * q4 + w
                        nc.tensor.matmul(
                            cs[:, PP * w:PP * (w + 1)],
                            ck_dm[hp][hrow:hrow + 64, 128 * j:128 * (j + 1)],
                            cq_dm[hp][hrow:hrow + 64, :],
                            start=True, stop=True)
                    ce = epool.tile([128, 1024], bf16, tag="e", name="ce")
                    nc.scalar.activation(ce[:], cs[:], AF.Exp)

                    def cav(item, last):
                        ce_, q4_ = item
                        for w in range(4):
                            j = 4 * q4_ + w
                            nc.tensor.matmul(
                                coT[:], cv_sb[j][:, 65 * hh:65 * hh + 65],
                                ce_[:, PP * w:PP * (w + 1)],
                                start=(q4_ == 0 and w == 0),
                                stop=(last and w == 3))
                    if pend is not None:
                        cav(pend, False)
                    pend = (ce, q4)
                cav(pend, True)
                rcp = small.tile([1, PP], fp32, tag="crcp", name="crcp")
                nc.scalar.copy(rcp[:], coT[64:65, :])
                rsum = small.tile([1, PP], fp32, tag="crsum", name="crsum")
                nc.vector.reciprocal_approx_fast(rsum[:], rcp[:])
                rbc = scratch.tile([64, PP], fp32, tag="crbc", name="crbc")
                nc.gpsimd.partition_broadcast(rbc[:], rsum[:], channels=64)
                nc.vector.tensor_tensor(out=co_dm[hp][hrow:hrow + 64, 0:PP],
                                        in0=coT[0:64, :], in1=rbc[:], op=OP.mult)
        coT_ps.release()
        cs_ps.release()

        # ---------- cwo + out ----------
        for pc in range(NPC):
            ps_ = proj3.tile([128, D], fp32, tag="pj3", name="pout")
            for c in range(DC):
                nc.tensor.matmul(ps_[:], co_dm[c][:, 128 * pc:128 * (pc + 1)],
                                 cwo_sb[c][:], start=(c == 0), stop=(c == DC - 1))
            ot = scratch.tile([128, D], fp32, tag="outt", name="outt")
            nc.vector.tensor_tensor(out=ot[:], in0=pooled[pc][:], in1=ps_[:],
                                    op=OP.add)
            nc.sync.dma_start(out_d[128 * pc:128 * (pc + 1), :], ot[:])

        proj3.release()
        plp.release()
        kvnrp.release()
        h3rp.release()
        wcr.release()
        rtmp.release()
        epool.release()
        opool.release()
        vpool.release()
        qpool.release()
        kpool.release()
        ownT.release()
        scratch.release()
        small.release()
        hnpool.release()
        hpool.release()
        const.release()

    nc.compile()
    return nc


_NC_CACHE = None


def kernel(**inputs):
    global _NC_CACHE
    in_maps = prepare_inputs(inputs)
    if _NC_CACHE is None:
        _NC_CACHE = build_nc()
    res = run_bass_kernel_spmd(_NC_CACHE, in_maps, core_ids=list(range(NCORES)))
    return assemble_output(res.results)
